# revision 13
# baseline (speedup 1.0000x reference)
"""Trainium2 Bass kernel for CRF layer loss + Viterbi decode.

Problem: B=1024, T=512, L=48.
  loss = sum_b [ logZ(b) - gold_score(b) ]    (forward logsumexp scan)
  pred_seq = Viterbi decode [B, T] int32

Sharding: data-parallel over batch, 128 sequences per core on 8 cores.

Per-core design (per time step t = 1..T-1):
  Forward (exp-space):  a_t[l,b] = (expT^T @ a_{t-1})[l,b] * exp(u_t)[l,b]
      PE matmul + one DVE mult; periodic renormalize (PE ones-matmul + recip),
      log-scales accumulated in lam[1,B]; logZ = lam + ln(sum_l a_final).
  Viterbi (exact max-plus on DVE, [b,(l,l')] layout):
      cand2 = nu ⊕ (W^T + eps*(47-l'))   (DVE, eps tie-break baked in)
      cand  = nu ⊕ W^T                   (GPSIMD, runs parallel to DVE)
      m2, m = grouped reduce_max         (DVE)
      bp = 47 - (m2-m)/eps  (exact index recovery), nu' = m + u_t
  Sequence scoring: one-hot STT gathers (+ PE one-hot matmul for W rows).
  Masking (t >= len): predicated-copy restore of frozen state.
  Backtrace: serial one-hot STT over stored bp (uint8 in SBUF).
"""

import numpy as np

B, T, L = 1024, 512, 48
NCORES = 8
BL = B // NCORES  # 128
EPS = 1.0 / 131072.0  # 2^-17 tie-break quantum
REBASE = 8            # forward renormalization period
CHUNK = 8             # time steps per DMA chunk

_CACHE = {}


def _build(t_steps=T, chunk=CHUNK, gp_cand=True, debug_bp=False):
    from concourse import bacc, tile, mybir

    F32 = mybir.dt.float32
    U8 = mybir.dt.uint8
    I32 = mybir.dt.int32
    ALU = mybir.AluOpType
    ACTF = mybir.ActivationFunctionType
    AXL = mybir.AxisListType

    assert t_steps % chunk == 0
    nchunks = t_steps // chunk

    nc = bacc.Bacc("TRN2", target_bir_lowering=False, debug=False)

    # ---- DRAM I/O ----
    u_d = nc.dram_tensor("u", [BL, t_steps, L], F32, kind="ExternalInput")
    tgtT_d = nc.dram_tensor("tgtT", [t_steps, BL], F32, kind="ExternalInput")
    tcol_d = nc.dram_tensor("tcol", [BL, t_steps], F32, kind="ExternalInput")
    imaskT_d = nc.dram_tensor("imaskT", [t_steps, BL], U8, kind="ExternalInput")
    maskT_d = nc.dram_tensor("maskT", [t_steps, BL], F32, kind="ExternalInput")
    imaskB_d = nc.dram_tensor("imaskB", [BL, t_steps], U8, kind="ExternalInput")
    maskB_d = nc.dram_tensor("maskB", [BL, t_steps], F32, kind="ExternalInput")
    expT_d = nc.dram_tensor("expT", [L, L], F32, kind="ExternalInput")
    w_d = nc.dram_tensor("w", [L, L], F32, kind="ExternalInput")
    wtb_d = nc.dram_tensor("wtb", [BL, L * L], F32, kind="ExternalInput")
    w2tb_d = nc.dram_tensor("w2tb", [BL, L * L], F32, kind="ExternalInput")
    iotap_d = nc.dram_tensor("iotap", [L, BL], F32, kind="ExternalInput")
    iota48_d = nc.dram_tensor("iota48", [BL, L], F32, kind="ExternalInput")
    iota48d_d = nc.dram_tensor("iota48d", [BL, L], F32, kind="ExternalInput")
    iota48u8_d = nc.dram_tensor("iota48u8", [BL, L], U8, kind="ExternalInput")
    ident_d = nc.dram_tensor("ident", [128, 128], F32, kind="ExternalInput")
    ones48_d = nc.dram_tensor("ones48", [L, 1], F32, kind="ExternalInput")
    ones1x48_d = nc.dram_tensor("ones1x48", [1, L], F32, kind="ExternalInput")

    lossrow_d = nc.dram_tensor("lossrow", [1, BL], F32, kind="ExternalOutput")
    seq_d = nc.dram_tensor("seq", [BL, t_steps], I32, kind="ExternalOutput")
    if debug_bp:
        bp_d = nc.dram_tensor("bp_dbg", [BL, t_steps * L], U8, kind="ExternalOutput")
        nvf_d = nc.dram_tensor("nvf_dbg", [BL, L], F32, kind="ExternalOutput")

    with tile.TileContext(nc) as tc:
        with (
            tc.tile_pool(name="persist", bufs=1) as pp,
            tc.tile_pool(name="chunks", bufs=3) as cp,
            tc.tile_pool(name="bigs", bufs=2) as bigp,
            tc.tile_pool(name="smalls", bufs=4) as sp,
            tc.tile_pool(name="fwd", bufs=3) as fp,
            tc.tile_pool(name="psA", bufs=2, space="PSUM") as psA,
            tc.tile_pool(name="psB", bufs=2, space="PSUM") as psB,
            tc.tile_pool(name="psC", bufs=1, space="PSUM") as psC,
            tc.tile_pool(name="psD", bufs=1, space="PSUM") as psD,
        ):
            # ---- persistent constants / state ----
            wtb = pp.tile([BL, L * L], F32)
            w2tb = pp.tile([BL, L * L], F32)
            expT = pp.tile([L, L], F32)
            w_sb = pp.tile([L, L], F32)
            iotap = pp.tile([L, BL], F32)
            iota48 = pp.tile([BL, L], F32)
            iota48d = pp.tile([BL, L], F32)
            iota48u8 = pp.tile([BL, L], U8)
            ident = pp.tile([128, 128], F32)
            ones48 = pp.tile([L, 1], F32)
            ones1x48 = pp.tile([1, L], F32)
            tcol = pp.tile([BL, t_steps], F32)
            imaskB = pp.tile([BL, t_steps], U8)
            maskB = pp.tile([BL, t_steps], F32)
            bp_store = pp.tile([BL, t_steps * L], U8)
            seq_sb = pp.tile([BL, t_steps], I32)
            lam = pp.tile([1, BL], F32)
            uacc = pp.tile([BL, 1], F32)
            tacc = pp.tile([BL, 1], F32)

            for dst, src in [
                (wtb, wtb_d), (w2tb, w2tb_d), (expT, expT_d), (w_sb, w_d),
                (iotap, iotap_d), (iota48, iota48_d), (iota48d, iota48d_d),
                (iota48u8, iota48u8_d), (ident, ident_d), (ones48, ones48_d),
                (ones1x48, ones1x48_d), (tcol, tcol_d), (imaskB, imaskB_d),
                (maskB, maskB_d),
            ]:
                nc.sync.dma_start(dst[:], src[:])
            nc.vector.memset(lam[:], 0.0)
            nc.vector.memset(uacc[:], 0.0)
            nc.vector.memset(tacc[:], 0.0)

            w2tb3 = w2tb[:].rearrange("p (a b) -> p a b", a=L)
            wtb3 = wtb[:].rearrange("p (a b) -> p a b", a=L)

            a_cur = None     # forward state [L, BL] exp-space
            nv_cur = None    # viterbi state [BL, L]
            pend_rb = None   # pending rebase broadcast (PSUM [L, BL])
            pend_ln = None   # pending ln(s) awaiting mask at t+1

            def load_chunk(c):
                t0 = c * chunk
                u_ch = cp.tile([BL, chunk * L], F32, tag="u_ch")
                nc.sync.dma_start(u_ch[:], u_d[:, t0:t0 + chunk, :])
                tgtrep = cp.tile([L, chunk * BL], F32, tag="tgtrep")
                nc.sync.dma_start(
                    tgtrep[:].rearrange("p (a b) -> p a b", a=chunk),
                    tgtT_d[t0:t0 + chunk, :].unsqueeze(0).partition_broadcast(L))
                imrep = cp.tile([L, chunk * BL], U8, tag="imrep")
                nc.sync.dma_start(
                    imrep[:].rearrange("p (a b) -> p a b", a=chunk),
                    imaskT_d[t0:t0 + chunk, :].unsqueeze(0).partition_broadcast(L))
                mrow = cp.tile([1, chunk * BL], F32, tag="mrow")
                nc.sync.dma_start(
                    mrow[:].rearrange("p (a b) -> p a b", a=chunk),
                    maskT_d[t0:t0 + chunk, :].unsqueeze(0))
                return u_ch, tgtrep, imrep, mrow

            def u_bl(u_ch, k):
                # u slice at in-chunk index k in [b, l] layout
                return u_ch[:, k * L:(k + 1) * L]

            def score_step(t, u_ch, tgtrep, k):
                # unary: u[b, t, tgt[b,t]], masked by maskB[:, t]
                usj = sp.tile([BL, L], F32, tag="usj")
                usv = sp.tile([BL, 1], F32, tag="usv")
                nc.vector.scalar_tensor_tensor(
                    usj[:], iota48[:], tcol[:, t:t + 1], u_bl(u_ch, k),
                    ALU.is_equal, ALU.mult, accum_out=usv[:])
                usm = sp.tile([BL, 1], F32, tag="usm")
                nc.vector.tensor_tensor(usm[:], usv[:], maskB[:, t:t + 1], ALU.mult)
                nc.vector.tensor_tensor(uacc[:], uacc[:], usm[:], ALU.add)
                # transition: W[tgt_t, tgt_{t+1}], masked by maskB[:, t+1]
                if t + 1 < t_steps:
                    oht = sp.tile([L, BL], F32, tag="oht")
                    nc.vector.tensor_tensor(
                        oht[:], iotap[:],
                        tgtrep[:, k * BL:(k + 1) * BL], ALU.is_equal)
                    wrow = psC.tile([BL, L], F32, tag="wrow")
                    nc.tensor.matmul(wrow[:], oht[:], w_sb[:])
                    tsj = sp.tile([BL, L], F32, tag="tsj")
                    tsv = sp.tile([BL, 1], F32, tag="tsv")
                    nc.vector.scalar_tensor_tensor(
                        tsj[:], iota48[:], tcol[:, t + 1:t + 2], wrow[:],
                        ALU.is_equal, ALU.mult, accum_out=tsv[:])
                    tsm = sp.tile([BL, 1], F32, tag="tsm")
                    nc.vector.tensor_tensor(tsm[:], tsv[:], maskB[:, t + 1:t + 2], ALU.mult)
                    nc.vector.tensor_tensor(tacc[:], tacc[:], tsm[:], ALU.add)

            # ================= time loop =================
            for c in range(nchunks):
                u_ch, tgtrep, imrep, mrow = load_chunk(c)
                for k in range(chunk):
                    t = c * chunk + k
                    if t == 0:
                        # init: a_0 = exp(u_0^T); nu_0 = u_0 - u_0[:,0]
                        tp0 = psB.tile([L, BL], F32, tag="tp")
                        nc.tensor.matmul(tp0[:], u_bl(u_ch, 0), ident[:],
                                         is_transpose=True)
                        a0 = fp.tile([L, BL], F32, tag="a")
                        nc.scalar.activation(a0[:], tp0[:], ACTF.Exp)
                        a_cur = a0
                        nv0 = sp.tile([BL, L], F32, tag="nv")
                        nc.vector.tensor_scalar(nv0[:], u_bl(u_ch, 0),
                                                u_ch[:, 0:1], None, ALU.subtract)
                        nv_cur = nv0
                        score_step(0, u_ch, tgtrep, 0)
                        continue

                    # lam += pending ln(s) * mask[t] (rebase at t-1 affects b
                    # iff b still alive at t; frozen b's scaling is undone by
                    # the predicated restore, so it must not count)
                    if pend_ln is not None:
                        lt1 = sp.tile([1, BL], F32, tag="lt1")
                        nc.vector.tensor_tensor(
                            lt1[:], pend_ln[:],
                            mrow[:, k * BL:(k + 1) * BL], ALU.mult)
                        nc.vector.tensor_tensor(lam[:], lam[:], lt1[:], ALU.add)
                        pend_ln = None

                    # ---------- forward pre (off critical chain) ----------
                    tp = psB.tile([L, BL], F32, tag="tp")
                    nc.tensor.matmul(tp[:], u_bl(u_ch, k), ident[:],
                                     is_transpose=True)
                    e_t = fp.tile([L, BL], F32, tag="e")
                    nc.scalar.activation(e_t[:], tp[:], ACTF.Exp)
                    if pend_rb is not None:
                        e2 = fp.tile([L, BL], F32, tag="e2")
                        nc.vector.tensor_tensor(e2[:], e_t[:], pend_rb[:], ALU.mult)
                        e_t = e2
                        pend_rb = None

                    # ---------- forward chain ----------
                    aps = psA.tile([L, BL], F32, tag="aps")
                    nc.tensor.matmul(aps[:], expT[:], a_cur[:])
                    a_new = fp.tile([L, BL], F32, tag="a")
                    nc.vector.tensor_tensor(a_new[:], aps[:], e_t[:], ALU.mult)
                    nc.vector.copy_predicated(
                        a_new[:], imrep[:, k * BL:(k + 1) * BL], a_cur[:])
                    a_cur = a_new

                    # ---------- forward rebase ----------
                    if t % REBASE == REBASE - 1 and t + 1 < t_steps:
                        s_ps = psD.tile([1, BL], F32, tag="s")
                        nc.tensor.matmul(s_ps[:], ones48[:], a_cur[:])
                        ln_s = sp.tile([1, BL], F32, tag="lns")
                        nc.scalar.activation(ln_s[:], s_ps[:], ACTF.Ln)
                        pend_ln = ln_s
                        rec = sp.tile([1, BL], F32, tag="rec")
                        nc.vector.reciprocal(rec[:], s_ps[:])
                        rb = psD.tile([L, BL], F32, tag="rb")
                        nc.tensor.matmul(rb[:], ones1x48[:], rec[:])
                        pend_rb = rb

                    # ---------- viterbi ----------
                    nv_b = nv_cur[:].unsqueeze(1).broadcast_to([BL, L, L])
                    cand2 = bigp.tile([BL, L * L], F32, tag="cand2")
                    nc.vector.tensor_tensor(
                        cand2[:].rearrange("p (a b) -> p a b", a=L),
                        nv_b, w2tb3, ALU.add)
                    candg = bigp.tile([BL, L * L], F32, tag="candg")
                    cand_eng = nc.gpsimd if gp_cand else nc.vector
                    cand_eng.tensor_tensor(
                        candg[:].rearrange("p (a b) -> p a b", a=L),
                        nv_b, wtb3, ALU.add)
                    m2 = sp.tile([BL, L], F32, tag="m2")
                    nc.vector.tensor_reduce(
                        m2[:], cand2[:].rearrange("p (a b) -> p a b", a=L),
                        AXL.X, ALU.max)
                    mr = sp.tile([BL, L], F32, tag="mr")
                    nc.vector.tensor_reduce(
                        mr[:], candg[:].rearrange("p (a b) -> p a b", a=L),
                        AXL.X, ALU.max)
                    d48 = sp.tile([BL, L], F32, tag="d48")
                    nc.vector.tensor_tensor(d48[:], m2[:], mr[:], ALU.subtract)
                    bpf = sp.tile([BL, L], F32, tag="bpf")
                    # +0.25 offset: exact under both truncating and
                    # round-to-nearest f32->u8 conversion (HW rounds, sim truncs)
                    nc.vector.tensor_scalar(bpf[:], d48[:], -1.0 / EPS, 47.25,
                                            ALU.mult, ALU.add)
                    bpc = sp.tile([BL, L], F32, tag="bpc")
                    nc.vector.tensor_scalar(bpc[:], bpf[:], 47.9, 0.0,
                                            ALU.min, ALU.max)
                    bps = bp_store[:, t * L:(t + 1) * L]
                    nc.vector.tensor_copy(bps, bpc[:])
                    nc.vector.copy_predicated(
                        bps, imaskB[:, t:t + 1].broadcast_to([BL, L]),
                        iota48u8[:])
                    nvn = sp.tile([BL, L], F32, tag="nvn")
                    nc.vector.tensor_tensor(nvn[:], mr[:], u_bl(u_ch, k), ALU.add)
                    nvr = sp.tile([BL, L], F32, tag="nv")
                    nc.vector.tensor_scalar(nvr[:], nvn[:], nvn[:, 0:1], None,
                                            ALU.subtract)
                    nc.vector.copy_predicated(
                        nvr[:], imaskB[:, t:t + 1].broadcast_to([BL, L]),
                        nv_cur[:])
                    nv_cur = nvr

                    # ---------- scores ----------
                    score_step(t, u_ch, tgtrep, k)

            # ================= finale =================
            # logZ row = lam + ln(sum_l a_final)
            sf = psD.tile([1, BL], F32, tag="s")
            nc.tensor.matmul(sf[:], ones48[:], a_cur[:])
            lnf = sp.tile([1, BL], F32, tag="lns")
            nc.scalar.activation(lnf[:], sf[:], ACTF.Ln)
            logz = pp.tile([1, BL], F32)
            nc.vector.tensor_tensor(logz[:], lam[:], lnf[:], ALU.add)
            # gold score, transposed to row
            sc = pp.tile([BL, 1], F32)
            nc.vector.tensor_tensor(sc[:], uacc[:], tacc[:], ALU.add)
            scT = psD.tile([1, BL], F32, tag="sct")
            nc.tensor.matmul(scT[:], sc[:], ident[:], is_transpose=True)
            lrow = pp.tile([1, BL], F32)
            nc.vector.tensor_tensor(lrow[:], logz[:], scT[:], ALU.subtract)
            nc.sync.dma_start(lossrow_d[:], lrow[:])

            # last tag = first-index argmax of nv_final
            mx = sp.tile([BL, 1], F32, tag="mx")
            nc.vector.tensor_reduce(mx[:], nv_cur[:], AXL.X, ALU.max)
            dd = sp.tile([BL, L], F32, tag="dd")
            nc.vector.scalar_tensor_tensor(dd[:], nv_cur[:], mx[:], iota48d[:],
                                           ALU.is_equal, ALU.mult)
            t2 = sp.tile([BL, 1], F32, tag="t2")
            nc.vector.tensor_reduce(t2[:], dd[:], AXL.X, ALU.max)
            tagf = pp.tile([BL, 1], F32)
            nc.vector.tensor_scalar(tagf[:], t2[:], -1.0, float(L), ALU.mult,
                                    ALU.add)

            # backtrace
            nc.vector.tensor_copy(seq_sb[:, t_steps - 1:t_steps], tagf[:])
            tag = tagf
            for t in range(t_steps - 1, 0, -1):
                btj = sp.tile([BL, L], F32, tag="btj")
                ntag = sp.tile([BL, 1], F32, tag="ntag")
                nc.vector.scalar_tensor_tensor(
                    btj[:], iota48[:], tag[:], bp_store[:, t * L:(t + 1) * L],
                    ALU.is_equal, ALU.mult, accum_out=ntag[:])
                nc.vector.tensor_copy(seq_sb[:, t - 1:t], ntag[:])
                tag = ntag

            nc.sync.dma_start(seq_d[:], seq_sb[:])
            if debug_bp:
                nc.sync.dma_start(bp_d[:], bp_store[:])
                nc.sync.dma_start(nvf_d[:], nv_cur[:])

    nc.finalize()
    return nc


def _get_nc(t_steps=T, chunk=CHUNK):
    key = (t_steps, chunk)
    if key not in _CACHE:
        _CACHE[key] = _build(t_steps, chunk)
    return _CACHE[key]


def _host_prep(inputs, targets, lens, transition, t_steps=T):
    """Build the 8 per-core input maps."""
    W = transition.astype(np.float64)
    lidx = np.arange(L)
    # wtb[l*48+l'] = W[l', l];  w2tb adds eps*(47-l')
    WT = W.T  # WT[l, l'] = W[l', l]
    wtb_flat = WT.reshape(-1).astype(np.float32)
    w2tb_flat = (WT + EPS * (47.0 - lidx)[None, :]).reshape(-1).astype(np.float32)
    consts = {
        "expT": np.exp(W).astype(np.float32),
        "w": W.astype(np.float32),
        "wtb": np.ascontiguousarray(np.broadcast_to(wtb_flat, (BL, L * L))),
        "w2tb": np.ascontiguousarray(np.broadcast_to(w2tb_flat, (BL, L * L))),
        "iotap": np.ascontiguousarray(
            np.broadcast_to(lidx[:, None].astype(np.float32), (L, BL))),
        "iota48": np.ascontiguousarray(
            np.broadcast_to(lidx[None, :].astype(np.float32), (BL, L))),
        "iota48d": np.ascontiguousarray(
            np.broadcast_to((L - lidx)[None, :].astype(np.float32), (BL, L))),
        "iota48u8": np.ascontiguousarray(
            np.broadcast_to(lidx[None, :].astype(np.uint8), (BL, L))),
        "ident": np.eye(128, dtype=np.float32),
        "ones48": np.ones((L, 1), np.float32),
        "ones1x48": np.ones((1, L), np.float32),
    }
    tpos = np.arange(t_steps)
    in_maps = []
    for c in range(NCORES):
        sl = slice(c * BL, (c + 1) * BL)
        u = np.ascontiguousarray(inputs[sl, :t_steps, :].astype(np.float32))
        tg = targets[sl, :t_steps].astype(np.float32)
        ln = lens[sl]
        mask = (tpos[None, :] < ln[:, None])  # [BL, t]
        m = {
            "u": u,
            "tgtT": np.ascontiguousarray(tg.T),
            "tcol": np.ascontiguousarray(tg),
            "maskT": np.ascontiguousarray(mask.T.astype(np.float32)),
            "imaskT": np.ascontiguousarray((~mask).T.astype(np.uint8)),
            "maskB": np.ascontiguousarray(mask.astype(np.float32)),
            "imaskB": np.ascontiguousarray((~mask).astype(np.uint8)),
        }
        m.update(consts)
        in_maps.append(m)
    return in_maps


def kernel(inputs, targets, lens, transition):
    from concourse.bass_utils import run_bass_kernel_spmd

    inputs = np.asarray(inputs, dtype=np.float32)
    targets = np.asarray(targets)
    lens = np.asarray(lens)
    transition = np.asarray(transition, dtype=np.float32)

    nc = _get_nc()
    in_maps = _host_prep(inputs, targets, lens, transition)
    res = run_bass_kernel_spmd(nc, in_maps, core_ids=list(range(NCORES)))

    loss = np.float32(0.0)
    seqs = []
    for c in range(NCORES):
        r = res.results[c]
        loss = loss + r["lossrow"].astype(np.float64).sum()
        seqs.append(r["seq"])
    pred_seq = np.concatenate(seqs, axis=0).astype(np.int32)
    return np.float32(loss), pred_seq
